# revision 21
# baseline (speedup 1.0000x reference)
"""Trainium2 Bass kernel for nn_BatchPitNorm1d (pairwise Gaussian-CDF KDE + inverse-normal).

Math:  u[b,f] = mean_s Phi((x[b,f] - c[s,f]) / bw[f]),  out = ndtri(u),
       bw = sigmoid(bw_param).

Algorithm (per core, data-parallel over batch):
  For fixed f, u is a smooth monotone function g_f(x) of x alone. Instead of
  B*S*F pairwise Phi evals, evaluate H_f(t) = ndtri(g_f(t)) at N=32 Chebyshev
  nodes t (4 nodes per core, the only O(S) work), AllGather the node values,
  fit a degree-31 Chebyshev polynomial per feature with one matmul, and
  evaluate it at the local x via Clenshaw. Truncation error ~1e-5, total
  error vs the f32 reference ~3.5e-4 max-abs (below the reference's own f32
  noise vs f64, ~6.8e-4).

Layout: features (F=128) on partitions everywhere.
"""

import math
from contextlib import ExitStack

import numpy as np

import concourse.bass as bass
import concourse.bacc as bacc
import concourse.tile as tile
from concourse import mybir
from concourse import bass_utils

F32 = mybir.dt.float32

N_CORES = 8
B, S, F = 512, 2048, 128
BL = B // N_CORES          # 64 batch rows per core
N_CHEB = 32                # Chebyshev nodes / polynomial order
NGRP = 4                   # node groups (cores 2g, 2g+1 share a node group)
NSPL = 2                   # sample splits (even core: half 0, odd: half 1)
NLOC = N_CHEB // NGRP      # 8 nodes per core
SL = S // NSPL             # 1024 samples per core
XDOM = 4.6                 # Chebyshev domain [-XDOM, XDOM] must cover all x
S_CHUNK = 128              # cdf_data DMA/transpose chunk (partition dim)

# Acklam's ndtri rational approximation (rel err ~1.2e-9 in exact arithmetic).
ACK_A = [-3.969683028665376e+01, 2.209460984245205e+02, -2.759285104469687e+02,
         1.383577518672690e+02, -3.066479806614716e+01, 2.506628277459239e+00]
ACK_B = [-5.447609879822406e+01, 1.615858368580409e+02, -1.556989798598866e+02,
         6.680131188771972e+01, -1.328068155288572e+01]
ACK_C = [-7.784894002430293e-03, -3.223964580411365e-01, -2.400758277161838e+00,
         -2.549732539343734e+00, 4.374664141464968e+00, 2.938163982698783e+00]
ACK_D = [7.784695709041462e-03, 3.224671290700398e-01, 2.445134137142996e+00,
         3.754408661907416e+00]
ACK_PLOW = 0.02425

# Tail branch: ndtri(v) = P((ln v - TAIL_C)/TAIL_H) for v in [1e-10, 0.0245],
# fitted offline (deg 12, max abs err 2.9e-6 in f32 Horner).
TAIL_C = -13.367466545685957
TAIL_H = 9.6583843842545
TAIL_P = [-4.6620662795522385, 1.987192922231145, 0.39130752786017514,
          0.15118687769960226, 0.07244870789988929, 0.03440053650770658,
          0.017071752063533348, 0.023374349987388514, 0.01857311946894496,
          -0.010751164259811812, -0.01069891020177563, 0.010969223349436171,
          0.00839706634883253]


def _cheb_nodes():
    th = (np.arange(N_CHEB) + 0.5) * np.pi / N_CHEB
    return (XDOM * np.cos(th)).astype(np.float32), th


def _fit_matrix():
    """Map H-at-nodes -> even/odd coefficients.

    Basis: T_j(w) (j<16) and xt*T_j(w) (j<16), w = 2*xt^2-1, xt = x/XDOM.
    Returns Cfit[n, k] with columns k: 0..15 = beta (even), 16..31 = gamma
    (odd), so alpha = H_nodes^T @ Cfit via the PE matmul.
    """
    _, th = _cheb_nodes()
    xt = np.cos(th)                      # normalized nodes
    w = 2 * xt * xt - 1
    J = N_CHEB // 2
    M = np.zeros((N_CHEB, N_CHEB))
    for j in range(J):
        M[:, j] = np.cos(j * np.arccos(np.clip(w, -1, 1)))
        M[:, J + j] = xt * M[:, j]
    Minv = np.linalg.inv(M)              # coeffs = Minv @ H
    return np.ascontiguousarray(Minv.T).astype(np.float32)


def _tt(nc, pool, in0, in1, op, name, tag=None):
    """Two-tensor op emitted as scalar_tensor_tensor (in0+0) op in1 —
    InstTensorScalarPtr supports the DVE 2x SBUF perf mode, InstTensorTensor
    does not."""
    t = pool.tile([in0.shape[0], in0.shape[1]], F32, name=name, tag=tag or name)
    nc.vector.scalar_tensor_tensor(out=t, in0=in0, scalar=0.0, in1=in1,
                                   op0=mybir.AluOpType.add, op1=op)
    return t


def _horner(nc, pool, r, coeffs, name):
    """Evaluate sum_j coeffs[j] * r^(J-1-j) via STT-fused Horner.

    acc_{j+1} = (acc_j + coeffs[j]) * r   [one scalar_tensor_tensor each],
    then a final tensor_scalar add of coeffs[-1].
    """
    p, w = r.shape[0], r.shape[1]
    acc = pool.tile([p, w], F32, name=f"{name}_h0", tag=f"{name}_h")
    nc.vector.tensor_scalar(out=acc, in0=r, scalar1=float(coeffs[0]), scalar2=None,
                            op0=mybir.AluOpType.mult)
    for j, cj in enumerate(coeffs[1:-1]):
        acc2 = pool.tile([p, w], F32, name=f"{name}_h{j + 1}", tag=f"{name}_h")
        nc.vector.scalar_tensor_tensor(out=acc2, in0=acc, scalar=float(cj),
                                       in1=r, op0=mybir.AluOpType.add,
                                       op1=mybir.AluOpType.mult)
        acc = acc2
    accf = pool.tile([p, w], F32, name=f"{name}_hf", tag=f"{name}_h")
    nc.vector.tensor_scalar(out=accf, in0=acc, scalar1=float(coeffs[-1]),
                            scalar2=None, op0=mybir.AluOpType.add)
    return accf


def _emit_ndtri(nc, pool, u, width, P=128):
    """Branchless Acklam ndtri on a [P, width] tile. Returns H tile."""
    one = 1.0

    # v = min(u, 1-u); sgn = 2*[u>=0.5]-1
    omu = pool.tile([P, width], F32, name="omu")
    nc.vector.tensor_scalar(out=omu, in0=u, scalar1=-1.0, scalar2=one,
                            op0=mybir.AluOpType.mult, op1=mybir.AluOpType.add)
    v0 = _tt(nc, pool, u, omu, mybir.AluOpType.min, "v0")
    v = pool.tile([P, width], F32, name="v")
    nc.vector.tensor_scalar(out=v, in0=v0, scalar1=1e-10, scalar2=None,
                            op0=mybir.AluOpType.max)
    mge = pool.tile([P, width], F32, name="mge")
    nc.vector.tensor_scalar(out=mge, in0=u, scalar1=0.5, scalar2=None,
                            op0=mybir.AluOpType.is_ge)
    sgn = pool.tile([P, width], F32, name="sgn")
    nc.vector.tensor_scalar(out=sgn, in0=mge, scalar1=2.0, scalar2=-1.0,
                            op0=mybir.AluOpType.mult, op1=mybir.AluOpType.add)

    # ---- central branch: q = u - 0.5, r = q^2
    q = pool.tile([P, width], F32, name="qc")
    nc.vector.tensor_scalar(out=q, in0=u, scalar1=-0.5, scalar2=None,
                            op0=mybir.AluOpType.add)
    r = _tt(nc, pool, q, q, mybir.AluOpType.mult, "rc")
    numc = _horner(nc, pool, r, ACK_A, "na")
    denc = _horner(nc, pool, r, ACK_B + [1.0], "da")
    dinv = pool.tile([P, width], F32, name="dinvc")
    nc.vector.reciprocal(out=dinv, in_=denc)
    nq = _tt(nc, pool, numc, q, mybir.AluOpType.mult, "nqc")
    xc = _tt(nc, pool, nq, dinv, mybir.AluOpType.mult, "xc")

    # ---- tail branch: polynomial in m = (ln v - C)/H  (no exp/sqrt needed)
    m = pool.tile([P, width], F32, name="mtail")
    nc.scalar.activation(out=m, in_=v, func=mybir.ActivationFunctionType.Ln,
                         accum_out=None)
    nc.vector.tensor_scalar(out=m, in0=m, scalar1=1.0 / TAIL_H,
                            scalar2=-TAIL_C / TAIL_H,
                            op0=mybir.AluOpType.mult, op1=mybir.AluOpType.add)
    rt = _horner(nc, pool, m, TAIL_P[::-1], "tp")
    # xt = -sgn * rt
    nsgn = pool.tile([P, width], F32, name="nsgn")
    nc.vector.tensor_scalar(out=nsgn, in0=sgn, scalar1=-1.0, scalar2=None,
                            op0=mybir.AluOpType.mult)
    xt = _tt(nc, pool, rt, nsgn, mybir.AluOpType.mult, "xt")

    # blend: h = xt + [v >= PLOW] * (xc - xt)
    mc = pool.tile([P, width], F32, name="mc")
    nc.vector.tensor_scalar(out=mc, in0=v, scalar1=float(ACK_PLOW), scalar2=None,
                            op0=mybir.AluOpType.is_ge)
    d = _tt(nc, pool, xc, xt, mybir.AluOpType.subtract, "dsel")
    md = _tt(nc, pool, mc, d, mybir.AluOpType.mult, "mdsel")
    h = _tt(nc, pool, xt, md, mybir.AluOpType.add, "hout")
    return h


def build(with_collective=True, stages=("load", "grid", "ndtri", "gather", "fit",
                                        "clenshaw", "store"), debug_taps=False):
    stages = set(stages)
    nc = bacc.Bacc("TRN2", target_bir_lowering=False, debug=False,
                   enable_asserts=False, num_devices=N_CORES)

    # Inputs arrive pre-transposed (feature-major) from the host shard step.
    x_t = nc.dram_tensor("x_t", [F, BL], F32, kind="ExternalInput")
    cdf_t = nc.dram_tensor("cdf_t", [F, SL], F32, kind="ExternalInput")
    bwp = nc.dram_tensor("bwp", [1, F], F32, kind="ExternalInput")
    tloc = nc.dram_tensor("tloc", [1, NLOC], F32, kind="ExternalInput")
    out = nc.dram_tensor("out", [F, BL], F32, kind="ExternalOutput")
    taps = {}
    if debug_taps:
        for nm, shp in [("d_gacc", [F, NLOC]), ("d_gsum", [N_CHEB, F]),
                        ("d_u", [N_CHEB, F]), ("d_h", [N_CHEB, F]),
                        ("d_alpha", [F, N_CHEB]), ("d_xt1", [F, BL])]:
            taps[nm] = nc.dram_tensor(nm, shp, F32, kind="ExternalOutput")

    cfit_h = nc.inline_tensor(_fit_matrix(), name="cfit")

    with tile.TileContext(nc) as tc, ExitStack() as ctx:
        io = ctx.enter_context(tc.tile_pool(name="io", bufs=2))
        small = ctx.enter_context(tc.tile_pool(name="small", bufs=1))
        nd = ctx.enter_context(tc.tile_pool(name="nd", bufs=2))
        psum = ctx.enter_context(tc.tile_pool(name="psum", bufs=2, space="PSUM"))
        dram = ctx.enter_context(tc.tile_pool(name="dram", bufs=1, space="DRAM"))

        # --- constants / small inputs
        cfit_sb = small.tile([N_CHEB, N_CHEB], F32)
        nc.sync.dma_start(out=cfit_sb, in_=cfit_h[:, :])
        bw_col = small.tile([F, 1], F32)
        nc.sync.dma_start(out=bw_col, in_=bwp.ap().rearrange("o f -> f o"))
        t_bc = small.tile([F, NLOC], F32)
        nc.sync.dma_start(
            out=t_bc,
            in_=bass.AP(tensor=tloc, offset=0, ap=[[0, F], [1, NLOC]]),
        )

        # --- bandwidth scalars: a = 1/(sigmoid(bwp)*sqrt(2)); neg_a = -a
        bw_sig = small.tile([F, 1], F32)
        nc.scalar.activation(out=bw_sig, in_=bw_col,
                             func=mybir.ActivationFunctionType.Sigmoid)
        inv_bw = small.tile([F, 1], F32)
        nc.vector.reciprocal(out=inv_bw, in_=bw_sig)
        a_col = small.tile([F, 1], F32)
        nc.vector.tensor_scalar(out=a_col, in0=inv_bw, scalar1=1.0 / math.sqrt(2.0),
                                scalar2=None, op0=mybir.AluOpType.mult)
        neg_a = small.tile([F, 1], F32)
        nc.vector.tensor_scalar(out=neg_a, in0=a_col, scalar1=-1.0,
                                scalar2=None, op0=mybir.AluOpType.mult)
        # bias_all[f, j] = a_f * t_j
        bias_all = small.tile([F, NLOC], F32)
        nc.vector.tensor_scalar_mul(out=bias_all, in0=t_bc, scalar1=a_col)

        # --- bulk loads (already feature-major; no transposes needed)
        cT = io.tile([F, SL], F32)
        if "load" in stages:
            nc.sync.dma_start(out=cT, in_=cdf_t[:, :])
        else:
            nc.vector.memset(cT, 0.0)
        x_sb = io.tile([F, BL], F32)
        nc.sync.dma_start(out=x_sb, in_=x_t[:, :])
        xt0 = small.tile([F, BL], F32)
        nc.vector.tensor_scalar(out=xt0, in0=x_sb, scalar1=1.0 / XDOM, scalar2=None,
                                op0=mybir.AluOpType.mult)
        xt1 = small.tile([F, BL], F32)  # clamp to [-1, 1]: off-domain x degrades
        nc.vector.tensor_scalar(out=xt1, in0=xt0, scalar1=1.0, scalar2=-1.0,
                                op0=mybir.AluOpType.min, op1=mybir.AluOpType.max)

        # --- grid pass: gacc[f, j] = sum_s erf(a_f * (t_j - c_sf)) over the
        # local sample half (ACT, the only O(S) work)
        gacc = small.tile([F, NLOC], F32)
        scratch = io.tile([128, SL], F32)
        if "grid" not in stages:
            nc.vector.memset(gacc, 0.0)
        for j in range(NLOC if "grid" in stages else 0):
            nc.scalar.activation(out=scratch, in_=cT,
                                 func=mybir.ActivationFunctionType.Erf,
                                 bias=bias_all[:, j:j + 1], scale=neg_a,
                                 accum_out=gacc[:, j:j + 1])

        # --- exchange: write gacc^T as [NLOC, F], AllGather (block order =
        # replica rank 2g + h), sum the two sample-halves -> g_sum [N, F]
        cin = dram.tile([NLOC, F], F32)
        g_sum = nd.tile([N_CHEB, F], F32)
        if "gather" in stages:
            nc.sync.dma_start(out=cin.rearrange("n f -> f n"), in_=gacc)
            cout = dram.tile([N_CORES, NLOC, F], F32,
                             addr_space="Shared" if with_collective else "Local")
            if with_collective:
                nc.gpsimd.collective_compute(
                    "AllGather", mybir.AluOpType.bypass,
                    replica_groups=[list(range(N_CORES))],
                    ins=[cin.opt()], outs=[cout.opt()],
                )
            gh = [nd.tile([N_CHEB, F], F32, name=f"gh{h}", tag=f"gh{h}")
                  for h in range(NSPL)]
            # readback: partition n = g*NLOC + row, skipping over the other half
            for h in range(NSPL):
                if with_collective:
                    src_ap = bass.AP(
                        tensor=cout.tensor, offset=cout.offset + h * NLOC * F,
                        ap=[[NSPL * NLOC * F, NGRP], [F, NLOC], [1, F]])
                else:  # stand-in: broadcast-read own block (timing model only)
                    src_ap = bass.AP(tensor=cin.tensor, offset=cin.offset,
                                     ap=[[0, NGRP], [F, NLOC], [1, F]])
                nc.sync.dma_start(out=gh[h][:, :], in_=src_ap)
            nc.vector.scalar_tensor_tensor(
                out=g_sum, in0=gh[0], scalar=0.0, in1=gh[1],
                op0=mybir.AluOpType.add, op1=mybir.AluOpType.add)
        else:
            nc.vector.memset(g_sum, 0.0)

        # u = 0.5 + g/(2S); H = ndtri(u) on [N, F] (every core, redundantly)
        u_nodes = nd.tile([N_CHEB, F], F32)
        nc.vector.tensor_scalar(out=u_nodes, in0=g_sum, scalar1=1.0 / (2.0 * S),
                                scalar2=0.5, op0=mybir.AluOpType.mult,
                                op1=mybir.AluOpType.add)
        if "ndtri" in stages:
            h_nodes = _emit_ndtri(nc, nd, u_nodes, F, P=N_CHEB)
        else:
            h_nodes = u_nodes

        # --- fit: alpha[f, k] = sum_n H[n, f] * Cfit[n, k]  (one matmul)
        alpha = small.tile([F, N_CHEB], F32)
        if "fit" in stages:
            alpha_ps = psum.tile([F, N_CHEB], F32, tag="mm")
            nc.tensor.matmul(out=alpha_ps, lhsT=h_nodes, rhs=cfit_sb,
                             start=True, stop=True)
            nc.vector.tensor_copy(out=alpha, in_=alpha_ps)
        else:
            nc.vector.memset(alpha, 0.0)

        # --- Clenshaw, even/odd split: y = pe(w) + xt*po(w), w = 2*xt^2-1.
        # Two independent all-STT chains pipeline on DVE without stalling on
        # the per-instruction write-ack.
        clen = ctx.enter_context(tc.tile_pool(name="clen", bufs=4))
        xsq = _tt(nc, clen, xt1, xt1, mybir.AluOpType.mult, "xsq")
        wt = clen.tile([F, BL], F32)
        nc.vector.tensor_scalar(out=wt, in0=xsq, scalar1=2.0, scalar2=-1.0,
                                op0=mybir.AluOpType.mult, op1=mybir.AluOpType.add)
        wt2 = clen.tile([F, BL], F32)
        nc.vector.tensor_scalar(out=wt2, in0=wt, scalar1=2.0, scalar2=None,
                                op0=mybir.AluOpType.mult)
        J = N_CHEB // 2
        nsteps = J - 1 if "clenshaw" in stages else 0

        def chain(name):
            b1 = clen.tile([F, BL], F32, name=f"{name}_b0", tag=f"{name}_b")
            nc.vector.memset(b1, 0.0)
            b2 = clen.tile([F, BL], F32, name=f"{name}_c0", tag=f"{name}_c")
            nc.vector.memset(b2, 0.0)
            return [b1, b2]

        ce = chain("ce"); co = chain("co")
        for j in range(nsteps, 0, -1):
            for name, ch, col in (("ce", ce, j), ("co", co, J + j)):
                b1, b2 = ch
                p = clen.tile([F, BL], F32, name=f"{name}_p{j}", tag=f"{name}_p")
                nc.vector.scalar_tensor_tensor(out=p, in0=b1, scalar=0.0, in1=wt2,
                                               op0=mybir.AluOpType.add,
                                               op1=mybir.AluOpType.mult)
                bn = clen.tile([F, BL], F32, name=f"{name}_b{j}", tag=f"{name}_b")
                nc.vector.scalar_tensor_tensor(out=bn, in0=p,
                                               scalar=alpha[:, col:col + 1],
                                               in1=b2, op0=mybir.AluOpType.add,
                                               op1=mybir.AluOpType.subtract)
                ch[1] = b1; ch[0] = bn
        # final step with w (not 2w)
        res = []
        for name, ch, col in (("ce", ce, 0), ("co", co, J)):
            b1, b2 = ch
            p = clen.tile([F, BL], F32, name=f"{name}_pf", tag=f"{name}_p")
            nc.vector.scalar_tensor_tensor(out=p, in0=b1, scalar=0.0, in1=wt,
                                           op0=mybir.AluOpType.add,
                                           op1=mybir.AluOpType.mult)
            r = clen.tile([F, BL], F32, name=f"{name}_r", tag=f"{name}_b")
            nc.vector.scalar_tensor_tensor(out=r, in0=p,
                                           scalar=alpha[:, col:col + 1],
                                           in1=b2, op0=mybir.AluOpType.add,
                                           op1=mybir.AluOpType.subtract)
            res.append(r)
        ye, yo = res
        xyo = _tt(nc, clen, yo, xt1, mybir.AluOpType.mult, "xyo")
        y = _tt(nc, clen, ye, xyo, mybir.AluOpType.add, "yfin")

        # --- store feature-major; the host un-transposes during gather
        nc.sync.dma_start(out=out[:, :], in_=y)
        if debug_taps:
            for h in range(NSPL):
                dt_ = nc.dram_tensor(f"d_gh{h}", [N_CHEB, F], F32,
                                     kind="ExternalOutput")
                nc.sync.dma_start(out=dt_[:, :], in_=gh[h])
            for nm, t in [("d_gacc", gacc), ("d_gsum", g_sum), ("d_u", u_nodes),
                          ("d_h", h_nodes), ("d_alpha", alpha), ("d_xt1", xt1)]:
                nc.sync.dma_start(out=taps[nm][:, :], in_=t)

    nc.compile()
    return nc


_CACHE = {}


def _get_nc():
    if "nc" not in _CACHE:
        _CACHE["nc"] = build(with_collective=True)
    return _CACHE["nc"]


def kernel(x, cdf_data, bw_param):
    x = np.ascontiguousarray(x, dtype=np.float32)
    cdf_data = np.ascontiguousarray(cdf_data, dtype=np.float32)
    bw_param = np.ascontiguousarray(bw_param, dtype=np.float32)
    nc = _get_nc()
    nodes, _ = _cheb_nodes()
    xt = np.ascontiguousarray(x.T)                      # [F, B]
    cdf_halves = [np.ascontiguousarray(cdf_data[h * SL:(h + 1) * SL].T)
                  for h in range(NSPL)]                  # each [F, SL]
    in_maps = []
    for i in range(N_CORES):
        g, h = i // NSPL, i % NSPL
        in_maps.append({
            "x_t": np.ascontiguousarray(xt[:, i * BL:(i + 1) * BL]),
            "cdf_t": cdf_halves[h],
            "bwp": bw_param,
            "tloc": nodes[g * NLOC:(g + 1) * NLOC].reshape(1, NLOC),
        })
    res = bass_utils.run_bass_kernel_spmd(nc, in_maps, core_ids=list(range(N_CORES)))
    return np.concatenate([res.results[i]["out"].T for i in range(N_CORES)], axis=0)
